# revision 24
# baseline (speedup 1.0000x reference)
"""ActionBindingAttention Trainium2 kernel.

Data-parallel over batch across 8 NeuronCores (256 items/core).

Host-side algebraic folds (all on small <=512x512 weights):
  - ident/k are batch-independent; q only feeds logits, so
    M = D**-0.5 * (ident @ Wk^T) @ Wq  [N, D] replaces the q projection.
  - LayerNorm mean is removed exactly by centering the action-encoder
    weights over the feature axis (a - mean(a) == actions_ext @ W_cent).
  - LN affine folds into Wv (Wv_eff = Wv * ln_g; b_v = ln_b @ Wv^T).
  - LN rsqrt(var) folds into the V-projection output copy (per-partition
    ACT scale).
  - sumsq for the LN variance comes from the Gram quadratic form
    sumsq = sum_a acts_ext * (G @ acts_ext), G = W_cent @ W_cent^T [65,65].
  - attention eps (1e-8, vs attn ~ 1/64) is far below the fp accuracy of
    the pipeline and is dropped.

Device layout per pair of batch items (i = 0, 1 stacked on partitions),
8 pairs per DMA group (HWDGE costs ~625ns per dma_start, serialized):
  v    [(i n)=128, g=512] = acts_ext @ (W_cent @ Wv_eff^T), x rinv (ACT copy)
  lg   [(i n)=128, s=64]  = MT_c^T @ slotsT_c  (PE, col-split tile_position)
  attn = softmax over s; Exp writes bf16 + accum_out gives the denominator
  as   [(i s)=128, g=512] = attn_i^T @ v_i     (PE, diagonal quadrants)
All activations restricted to one ACT table set (exp/ln/copy); rsqrt is
Exp(-0.5*Ln(x)) to avoid per-pair table reloads.
"""

import numpy as np
import ml_dtypes

import concourse.bass as bass
import concourse.bacc as bacc
import concourse.mybir as mybir
import concourse.tile as tile
from concourse.bass_utils import run_bass_kernel_spmd

B, S, D = 2048, 64, 512
N, A, F = 64, 64, 512
LN_EPS = 1e-5
NCORES = 8
PER = B // NCORES          # 256 items per core
PAIRS = PER // 2           # 128

BF16 = mybir.dt.bfloat16
F32 = mybir.dt.float32
AF = mybir.ActivationFunctionType


def _patch_act_tables():
    # Force every activation onto the one table that holds exp+ln+copy so the
    # per-pair Exp/Ln never alternates table loads (256 table DMAs dominated
    # the first HW runs). Keep dict length/order so act_func_set_ids stay
    # valid; just make every other set ineligible.
    import concourse.bacc as _bacc

    if getattr(_bacc, "_act_tables_patched", False):
        return
    orig = _bacc.get_activation_tables

    def patched(arch):
        tabs = orig(arch)
        keep = "natural_log_exp_and_others"
        return {k: (v if k == keep else set()) for k, v in tabs.items()}

    _bacc.get_activation_tables = patched
    _bacc._act_tables_patched = True


def _build(add_bv: bool, pairs: int = PAIRS):
    _patch_act_tables()
    G = 8 if pairs % 8 == 0 else 1          # pairs per DMA group
    ngroups = pairs // G
    nc = bacc.Bacc("TRN2")
    acts = nc.declare_dram_parameter("acts_ext", [A + 1, pairs, 2 * N], BF16, isOutput=False)
    slt = nc.declare_dram_parameter("slotsT", [128, pairs, 4, 2, S], BF16, isOutput=False)
    wfold = nc.declare_dram_parameter("wfold", [A + 1, F], BF16, isOutput=False)
    mt = nc.declare_dram_parameter("mt", [4, 128, 2 * N], BF16, isOutput=False)
    gm = nc.declare_dram_parameter("gram", [A + 1, A + 1], BF16, isOutput=False)
    if add_bv:
        bvp = nc.declare_dram_parameter("bv", [1, F], F32, isOutput=False)
    out_as = nc.declare_dram_parameter("out_aslots", [128, pairs, F], BF16, isOutput=True)
    out_at = nc.declare_dram_parameter("out_attn", [128, pairs, S], BF16, isOutput=True)

    with tile.TileContext(nc) as tc:
        with (
            tc.tile_pool(name="consts", bufs=1) as cpool,
            tc.tile_pool(name="io", bufs=4) as io,
            tc.tile_pool(name="wk", bufs=3) as wk,
            tc.tile_pool(name="ps_g", bufs=1, space=bass.MemorySpace.PSUM) as ps_g_p,
            tc.tile_pool(name="ps_ss", bufs=1, space=bass.MemorySpace.PSUM) as ps_ss_p,
            tc.tile_pool(name="ps_v", bufs=2, space=bass.MemorySpace.PSUM) as ps_v_p,
            tc.tile_pool(name="ps_lg", bufs=2, space=bass.MemorySpace.PSUM) as ps_lg_p,
            tc.tile_pool(name="ps_as", bufs=2, space=bass.MemorySpace.PSUM) as ps_as_p,
        ):
            wfold_sb = cpool.tile([A + 1, F], BF16)
            nc.sync.dma_start(wfold_sb[:], wfold[:])
            mt_sb = cpool.tile([128, 4, 2 * N], BF16)
            nc.sync.dma_start(mt_sb[:], mt.rearrange("c p j -> p c j"))
            g_sb = cpool.tile([A + 1, A + 1], BF16)
            nc.sync.dma_start(g_sb[:], gm[:])
            ones_sb = cpool.tile([A + 1, 1], BF16)
            nc.gpsimd.memset(ones_sb[:], 1.0)
            eps_sb = cpool.tile([128, 1], F32)
            nc.gpsimd.memset(eps_sb[:], LN_EPS)
            zero_sb = cpool.tile([128, 1], F32)
            nc.gpsimd.memset(zero_sb[:], 0.0)
            if add_bv:
                bv_sb = cpool.tile([1, F], F32)
                nc.sync.dma_start(bv_sb[:], bvp[:])

            for g in range(ngroups):
                p0 = g * G
                acts_t = io.tile([A + 1, G, 2 * N], BF16, tag="acts")
                nc.sync.dma_start(acts_t[:], acts[:, p0 : p0 + G, :])
                sl_t = io.tile([128, G, 4, 2, S], BF16, tag="sl")
                nc.sync.dma_start(sl_t[:], slt[:, p0 : p0 + G, :, :, :])
                as_g = wk.tile([128, G, F], BF16, tag="ass")
                at_g = wk.tile([128, G, S], BF16, tag="attnb")

                for j in range(G):
                    p = p0 + j
                    a_j = acts_t[:, j, :]

                    # LN variance via Gram quadratic form
                    ps_g = ps_g_p.tile([A + 1, 128], F32, tag="g")
                    nc.tensor.matmul(ps_g[:], g_sb[:], a_j, start=True, stop=True)
                    prod = wk.tile([A + 1, 128], BF16, tag="prod")
                    nc.vector.tensor_tensor(
                        prod[:], a_j, ps_g[:], op=mybir.AluOpType.mult
                    )
                    ps_ss = ps_ss_p.tile([128, 1], F32, tag="ss")
                    nc.tensor.matmul(
                        ps_ss[:], prod[:], ones_sb[:], start=True, stop=True
                    )
                    lnv = wk.tile([128, 1], F32, tag="lnv")
                    nc.scalar.activation(
                        lnv[:], ps_ss[:], AF.Ln, scale=1.0 / F, bias=eps_sb[:]
                    )
                    rinv = wk.tile([128, 1], F32, tag="rinv")
                    nc.scalar.activation(
                        rinv[:], lnv[:], AF.Exp, scale=-0.5, bias=zero_sb[:]
                    )

                    # V projection: LN linear given r -> acts @ (W_cent @ Wv_eff^T)
                    ps_v = ps_v_p.tile([128, F], F32, tag="v")
                    nc.tensor.matmul(
                        ps_v[:], a_j, wfold_sb[:], start=True, stop=True
                    )
                    v_sb = wk.tile([128, F], BF16, tag="vs")
                    nc.scalar.activation(v_sb[:], ps_v[:], AF.Copy, scale=rinv[:])
                    if add_bv:
                        nc.vector.tensor_tensor(
                            v_sb[:], v_sb[:], bv_sb[:].to_broadcast([128, F]),
                            op=mybir.AluOpType.add,
                        )

                    # logits via M @ slotsT, col-split so softmax is per-partition
                    ps_lg = ps_lg_p.tile([128, S], F32, tag="lg")
                    for i in (0, 1):
                        for c in range(4):
                            nc.tensor.matmul(
                                ps_lg[i * 64 : (i + 1) * 64, :],
                                mt_sb[:, c, i * 64 : (i + 1) * 64],
                                sl_t[:, j, c, i, :],
                                start=(c == 0),
                                stop=(c == 3),
                                tile_position=(0, i * 64),
                            )
                    e_sb = wk.tile([128, S], BF16, tag="e")
                    ssum = wk.tile([128, 1], F32, tag="ssum")
                    nc.scalar.activation(
                        e_sb[:], ps_lg[:], AF.Exp, bias=zero_sb[:],
                        accum_out=ssum[:],
                    )
                    rcp = wk.tile([128, 1], F32, tag="rcp")
                    nc.vector.reciprocal(rcp[:], ssum[:])
                    nc.vector.tensor_scalar_mul(at_g[:, j, :], e_sb[:], rcp[:])

                    # weighted sum: both items in one psum via diagonal quadrants
                    ps_as = ps_as_p.tile([128, F], F32, tag="as")
                    for i in (0, 1):
                        nc.tensor.matmul(
                            ps_as[i * 64 : (i + 1) * 64, :],
                            at_g[i * 64 : (i + 1) * 64, j, :],
                            v_sb[i * 64 : (i + 1) * 64, :],
                            start=True,
                            stop=True,
                            tile_position=(i * 64, i * 64),
                        )
                    nc.vector.tensor_copy(as_g[:, j, :], ps_as[:])

                nc.sync.dma_start(out_as[:, p0 : p0 + G, :], as_g[:])
                nc.sync.dma_start(out_at[:, p0 : p0 + G, :], at_g[:])
    nc.compile()
    return nc


def _prepare(slots, actions, W_ae, b_ae, W_ie, b_ie, ln_g, ln_b, Wq, Wk, Wv):
    slots = np.asarray(slots, np.float32)
    actions = np.asarray(actions, np.float32)
    f64 = lambda x: np.asarray(x, np.float64)
    bf16 = lambda x: np.asarray(x).astype(ml_dtypes.bfloat16)

    # batch-independent folds (host, float64)
    ident = f64(W_ie).T + f64(b_ie)[None, :]                  # [N, F]
    kmat = ident @ f64(Wk).T                                  # [N, F]
    M = (D ** -0.5) * (kmat @ f64(Wq))                        # [N, D]
    mtT = M.T                                                 # [D, N]
    mt = np.empty((4, 128, 2 * N))
    for c in range(4):
        mt[c, :, :N] = mtT[c * 128 : (c + 1) * 128]
        mt[c, :, N:] = mtT[c * 128 : (c + 1) * 128]
    w_ext = np.concatenate([f64(W_ae).T, f64(b_ae)[None, :]], axis=0)  # [65, F]
    w_cent = w_ext - w_ext.mean(axis=1, keepdims=True)
    gram = w_cent @ w_cent.T                                  # [65, 65]
    wv_eff = f64(Wv) * f64(ln_g)[None, :]                     # [g, f]
    wfold = w_cent @ wv_eff.T                                 # [65, g]
    b_v = (f64(ln_b) @ f64(Wv).T).astype(np.float32)          # [F]
    add_bv = bool(np.any(b_v != 0.0))

    # partition-major layouts: [partition, pair, ...] per core shard
    # acts_ext[a, p, i*64+n] = actions[2p+i, n, a]; row A = 1.0
    acts_ext = np.empty((A + 1, B // 2, 2 * N), np.float32)
    acts_ext[:A] = (
        actions.reshape(B // 2, 2, N, A).transpose(3, 0, 1, 2).reshape(A, B // 2, 2 * N)
    )
    acts_ext[A] = 1.0
    # slotsT[dp, p, c, i, s] = slots[2p+i, s, c*128+dp]
    slotsT = np.ascontiguousarray(
        slots.reshape(B // 2, 2, S, 4, 128).transpose(4, 0, 3, 1, 2)
    ).reshape(128, B // 2, 4, 2, S)

    consts = {
        "wfold": bf16(wfold),
        "mt": bf16(mt),
        "gram": bf16(gram),
    }
    if add_bv:
        consts["bv"] = b_v[None, :]
    acts_b = bf16(acts_ext)
    slots_b = bf16(slotsT)
    in_maps = []
    for c in range(NCORES):
        lo, hi = c * (PER // 2), (c + 1) * (PER // 2)
        in_maps.append(
            {
                "acts_ext": np.ascontiguousarray(acts_b[:, lo:hi]),
                "slotsT": np.ascontiguousarray(slots_b[:, lo:hi]),
                **consts,
            }
        )

    nc = _build(add_bv)
    return nc, in_maps


def _gather(res):
    # outputs are [128=(i,x), pairs, F/S]; reassemble to [B, ...]
    aslots = np.concatenate(
        [
            r["out_aslots"].astype(np.float32)
            .reshape(2, S, PER // 2, F).transpose(2, 0, 1, 3).reshape(PER, S, F)
            for r in res.results
        ],
        axis=0,
    )
    attn = np.concatenate(
        [
            r["out_attn"].astype(np.float32)
            .reshape(2, N, PER // 2, S).transpose(2, 0, 1, 3).reshape(PER, N, S)
            for r in res.results
        ],
        axis=0,
    )
    return aslots, attn


def kernel(**inputs):
    nc, in_maps = _prepare(**inputs)
    res = run_bass_kernel_spmd(nc, in_maps, list(range(NCORES)), trace=False)
    return _gather(res)
